# revision 11
# baseline (speedup 1.0000x reference)
"""Decorrelated (whitening) group norm for Trainium2, 8 NeuronCores.

Problem: x (16, 64, 224, 224) f32; G=32 groups where group(channel-row r) = r % 32
(after flattening batch*channel to 1024 rows). Whitening: y = sigma^{-1/2} (x - mean)
per group, sigma the 32x32 group covariance.

Strategy (single NEFF, SPMD on 8 cores, data-parallel over batch):
  - Core k gets rows [128k, 128k+128) as a (128, 50176) f32 tensor; row p is group
    p % 32 (4 group-blocks of 32 on the partition axis). Whole shard stays resident
    in SBUF (~196 KiB/partition).
  - Pass 1: for each 128-col chunk: PE-transpose (fp32) -> PSUM, copy-cast to bf16
    SBUF (alternating DVE/ACT), then accumulate gram (128x128) = sum T^T T and row
    sums via two PE matmuls into persistent PSUM banks.
  - Block-reduce gram/sums to (32, 33) with tiny selection matmuls; AllReduce the
    4 KB partial over the 8 cores (TOPSP collective).
  - On-device 32x32 math: sigma = S/n - mean mean^T + eps I; Newton-Schulz
    iterations give wm = sigma^{-1/2} (equals the reference's SVD-based result to
    fp32 precision since sigma's eigenvalues are ~1).
  - Pass 2: y = x + bdiag(wm - I) @ x - wm@mean. The exact +x and per-partition bias are fused into one
    DVE scalar_tensor_tensor during PSUM eviction; DMA out.
"""

import functools
import os
import sys

import numpy as np

if "/opt/trn_rl_repo" not in sys.path:
    sys.path.insert(0, "/opt/trn_rl_repo")

B, C, H, W = 16, 64, 224, 224
G = 32
EPS = 1e-5
NCORES = 8
ROWS = 128                 # per-core rows = 2 batches * 64 channels
COLS = H * W               # 50176
NS_ITERS = 6

LAST_RESULTS = None        # BassKernelResults of the most recent run (for test harness)


@functools.lru_cache(maxsize=4)
def _build(cols, ncores, f32r_tpose=False, f32r_apply=False, lookahead=2):
    import concourse.bass as bass
    import concourse.tile as tile
    from concourse import bacc, mybir

    f32 = mybir.dt.float32
    f32r = mybir.dt.float32r
    bf16 = mybir.dt.bfloat16
    ADD = mybir.AluOpType.add

    ntot = 4 * cols * ncores          # elements per group, all cores
    ntch = cols // 128                # transpose chunks
    nach = cols // 512                # apply chunks
    nlch = 16                         # load DMAs
    assert cols % 1024 == 0 and cols % 128 == 0 and cols % nlch == 0

    nc = bacc.Bacc(
        "TRN2", target_bir_lowering=False, debug=False, num_devices=ncores
    )
    xin = nc.dram_tensor("x", [ROWS, cols], f32, kind="ExternalInput")
    yout = nc.dram_tensor("y", [ROWS, cols], f32, kind="ExternalOutput")
    xin_ap = xin.ap()
    yout_ap = yout.ap()

    i128_d = nc.inline_tensor(np.eye(128, dtype=np.float32), name="i128c")
    p4_d = nc.inline_tensor(
        np.tile(np.eye(32, dtype=np.float32), (4, 1)), name="p4c"
    )

    def rcast(ap):
        # tags for the pass-1 transpose chain (loads / identity / transpose)
        return ap.bitcast(f32r) if f32r_tpose else ap

    def acast(ap):
        # tags for the pass-2 apply matmul
        return ap.bitcast(f32r) if f32r_apply else ap

    with tile.TileContext(nc) as tc:
        with (
            tc.tile_pool(name="consts", bufs=1) as consts,
            tc.tile_pool(name="xpool", bufs=1) as xpool,
            tc.tile_pool(name="smalls", bufs=1) as smalls,
        ):
            # ---- constants ----
            # i128's DMA is f32r-tagged so it can feed the f32r PE transpose as
            # the identity; fp32 matmuls and DVE ops read it via plain-f32 APs.
            i128 = consts.tile([128, 128], f32, name="i128")
            nc.sync.dma_start(rcast(i128[:]), rcast(i128_d.ap()))
            p4 = consts.tile([128, 32], f32, name="p4")
            nc.sync.dma_start(p4[:], p4_d.ap())
            ones_bf = consts.tile([128, 1], bf16, name="ones_bf")
            nc.vector.memset(ones_bf[:], 1.0)
            ones32f = consts.tile([32, 1], f32, name="ones32f")
            nc.vector.memset(ones32f[:], 1.0)
            onesrow = consts.tile([1, 32], f32, name="onesrow")
            nc.vector.memset(onesrow[:], 1.0)
            c15I = consts.tile([32, 32], f32, name="c15I")
            nc.vector.tensor_scalar_mul(c15I[:], i128[0:32, 0:32], 1.5)
            epsI = consts.tile([32, 32], f32, name="epsI")
            nc.vector.tensor_scalar_mul(epsI[:], i128[0:32, 0:32], EPS)

            # ---- resident shard load ----
            xres = xpool.tile([128, cols], f32, name="xres")
            lch = cols // nlch
            for k in range(nlch):
                nc.sync.dma_start(
                    rcast(xres[:, k * lch:(k + 1) * lch]),
                    rcast(xin_ap[:, k * lch:(k + 1) * lch]),
                )

            # ---- pass 1: gram + sums accumulation ----
            with (
                tc.tile_pool(name="psA", bufs=4, space="PSUM") as psA,
                tc.tile_pool(name="psAcc", bufs=1, space="PSUM") as psAcc,
                tc.tile_pool(name="tstage", bufs=4) as tstage,
            ):
                gramP = psAcc.tile([128, 128], f32, name="gramP")
                s1P = psAcc.tile([128, 1], f32, name="s1P")

                pend = {}

                def emit_transpose(j):
                    tp = psA.tile([128, 128], f32, name="tp")
                    nc.tensor.transpose(
                        rcast(tp[:]),
                        rcast(xres[:, j * 128:(j + 1) * 128]),
                        rcast(i128[:]),
                    )
                    tb = tstage.tile([128, 128], bf16, name="tb")
                    if j % 2 == 0:
                        nc.vector.tensor_copy(tb[:], tp[:])
                    else:
                        nc.scalar.copy(tb[:], tp[:])
                    pend[j] = tb

                def emit_cov(j):
                    tb = pend.pop(j)
                    nc.tensor.matmul(
                        gramP[:], tb[:], tb[:],
                        start=(j == 0), stop=(j == ntch - 1),
                    )
                    nc.tensor.matmul(
                        s1P[:], tb[:], ones_bf[:],
                        start=(j == 0), stop=(j == ntch - 1),
                    )

                la = min(lookahead, ntch)
                for j in range(ntch):
                    emit_transpose(j)
                    if j >= la:
                        emit_cov(j - la)
                for j in range(ntch - la, ntch):
                    emit_cov(j)

                # evict accumulators while their banks are still in scope
                gram_sb = smalls.tile([128, 128], f32, name="gram_sb")
                nc.vector.tensor_copy(gram_sb[:], gramP[:])
                s1_sb = smalls.tile([128, 1], f32, name="s1_sb")
                nc.vector.tensor_copy(s1_sb[:], s1P[:])

            # ---- block reduce to (32, 33) + allreduce ----
            with tc.tile_pool(name="psS", bufs=3, space="PSUM") as psS:
                sigP = psS.tile([32, 32], f32, name="psml")
                for i in range(4):
                    nc.tensor.matmul(
                        sigP[:],
                        i128[:, 32 * i:32 * (i + 1)],
                        gram_sb[:, 32 * i:32 * (i + 1)],
                        start=(i == 0), stop=(i == 3),
                    )
                s1rP = psS.tile([32, 1], f32, name="psml")
                nc.tensor.matmul(s1rP[:], p4[:], s1_sb[:], start=True, stop=True)

                part = smalls.tile([32, 33], f32, name="part")
                nc.vector.tensor_copy(part[:, 0:32], sigP[:])
                nc.vector.tensor_copy(part[:, 32:33], s1rP[:])

                with tc.tile_pool(name="dram", bufs=1, space="DRAM") as dram:
                    cin = dram.tile([32, 33], f32, name="cc_in")
                    cout = dram.tile([32, 33], f32, name="cc_out")
                    nc.sync.dma_start(cin[:], part[:])
                    nc.gpsimd.collective_compute(
                        "AllReduce",
                        mybir.AluOpType.add,
                        replica_groups=[list(range(ncores))],
                        ins=[cin.opt()],
                        outs=[cout.opt()],
                    )
                    ar = smalls.tile([32, 33], f32, name="ar")
                    nc.sync.dma_start(ar[:], cout[:])

                # ---- tiny stats math ----
                inv_n = 1.0 / float(ntot)
                mean = smalls.tile([32, 1], f32, name="mean")
                nc.vector.tensor_scalar_mul(mean[:], ar[:, 32:33], inv_n)
                sig0 = smalls.tile([32, 32], f32, name="sig0")
                nc.vector.tensor_scalar_mul(sig0[:], ar[:, 0:32], inv_n)

                # mean row (1,32) via PE transpose
                mrowP = psS.tile([1, 32], f32, name="psml")
                nc.tensor.transpose(mrowP[:], mean[:], i128[0:32, 0:32])
                mrow = smalls.tile([1, 32], f32, name="mrow")
                nc.vector.tensor_copy(mrow[:], mrowP[:])
                outerP = psS.tile([32, 32], f32, name="psml")
                nc.tensor.matmul(outerP[:], mrow[:], mrow[:], start=True, stop=True)

                sigma = smalls.tile([32, 32], f32, name="sigma")
                # sigma = sig0 - outer
                nc.vector.scalar_tensor_tensor(
                    out=sigma[:], in0=outerP[:], scalar=-1.0, in1=sig0[:],
                    op0=mybir.AluOpType.mult, op1=ADD,
                )
                nc.vector.tensor_add(sigma[:], sigma[:], epsI[:])

                # t = trace(sigma)/32, per-partition scalars rt = 1/t, rs = t^-1/2
                diag = smalls.tile([32, 32], f32, name="diag")
                nc.vector.tensor_mul(diag[:], sigma[:], i128[0:32, 0:32])
                dvec = smalls.tile([32, 1], f32, name="dvec")
                nc.vector.reduce_sum(dvec[:], diag[:], axis=mybir.AxisListType.X)
                trP = psS.tile([1, 1], f32, name="psml")
                nc.tensor.matmul(trP[:], dvec[:], ones32f[:], start=True, stop=True)
                tr_sb = smalls.tile([1, 1], f32, name="tr_sb")
                nc.vector.tensor_copy(tr_sb[:], trP[:])
                tr4P = psS.tile([32, 1], f32, name="psml")
                nc.tensor.matmul(tr4P[:], onesrow[:], tr_sb[:], start=True, stop=True)
                tA = smalls.tile([32, 1], f32, name="tA")
                nc.vector.tensor_scalar_mul(tA[:], tr4P[:], 1.0 / 32.0)
                rt = smalls.tile([32, 1], f32, name="rt")
                nc.vector.reciprocal(rt[:], tA[:])
                rs = smalls.tile([32, 1], f32, name="rs")
                nc.scalar.activation(
                    rs[:], rt[:], mybir.ActivationFunctionType.Sqrt
                )

                A = smalls.tile([32, 32], f32, name="A")
                nc.vector.tensor_scalar_mul(A[:], sigma[:], rt[:])

                # ---- Newton-Schulz: Y0 = A, Z0 = I ----
                Y = smalls.tile([32, 32], f32, name="Y")
                nc.vector.tensor_copy(Y[:], A[:])
                Z = smalls.tile([32, 32], f32, name="Z")
                nc.vector.tensor_copy(Z[:], i128[0:32, 0:32])
                with tc.tile_pool(name="nsbuf", bufs=2) as nsbuf:
                    for _ in range(NS_ITERS):
                        zyP = psS.tile([32, 32], f32, name="psml")
                        nc.tensor.matmul(zyP[:], Z[:], Y[:], start=True, stop=True)
                        Wt = nsbuf.tile([32, 32], f32, name="Wt")
                        nc.vector.scalar_tensor_tensor(
                            out=Wt[:], in0=zyP[:], scalar=-0.5, in1=c15I[:],
                            op0=mybir.AluOpType.mult, op1=ADD,
                        )
                        ypP = psS.tile([32, 32], f32, name="psml")
                        nc.tensor.matmul(ypP[:], Y[:], Wt[:], start=True, stop=True)
                        zpP = psS.tile([32, 32], f32, name="psml")
                        nc.tensor.matmul(zpP[:], Wt[:], Z[:], start=True, stop=True)
                        nc.vector.tensor_copy(Y[:], ypP[:])
                        nc.vector.tensor_copy(Z[:], zpP[:])

                # wm = Z / sqrt(t); R = wm - I; negb = -wm @ mean
                wm = smalls.tile([32, 32], f32, name="wm")
                nc.vector.tensor_scalar_mul(wm[:], Z[:], rs[:])
                R = smalls.tile([32, 32], f32, name="R")
                nc.vector.tensor_sub(R[:], wm[:], i128[0:32, 0:32])
                bP = psS.tile([32, 1], f32, name="psml")
                nc.tensor.matmul(bP[:], wm[:], mean[:], start=True, stop=True)
                negb = smalls.tile([32, 1], f32, name="negb")
                nc.vector.tensor_scalar_mul(negb[:], bP[:], -1.0)

            # broadcast to 128 partitions: b4, WM4R = bdiag(R)
            b4 = smalls.tile([128, 1], f32, name="b4")
            wm4r_f = smalls.tile([128, 128], f32, name="wm4r_f")
            nc.vector.memset(wm4r_f[:], 0.0)
            for i in range(4):
                nc.sync.dma_start(b4[32 * i:32 * (i + 1), :], negb[:])
                nc.sync.dma_start(
                    wm4r_f[32 * i:32 * (i + 1), 32 * i:32 * (i + 1)], R[:]
                )
            if f32r_apply:
                # rounding copy so the f32r apply-matmul sees an f32r producer
                wm4r = smalls.tile([128, 128], f32, name="wm4r")
                nc.vector.tensor_copy(acast(wm4r[:]), wm4r_f[:])
            else:
                wm4r = wm4r_f

            # ---- pass 2: y = x + bdiag(R) x + b4 ----
            with (
                tc.tile_pool(name="psY", bufs=3, space="PSUM") as psY,
                tc.tile_pool(name="ystage", bufs=4) as ystage,
            ):
                for c in range(nach):
                    c0 = c * 512
                    yP = psY.tile([128, 512], f32, name="yP")
                    nc.tensor.matmul(
                        yP[:],
                        acast(wm4r[:]),
                        acast(xres[:, c0:c0 + 512]),
                        start=True, stop=True,
                    )
                    yb = ystage.tile([128, 512], f32, name="yb")
                    nc.vector.scalar_tensor_tensor(
                        out=yb[:],
                        in0=yP[:],
                        scalar=b4[:],
                        in1=xres[:, c0:c0 + 512],
                        op0=ADD, op1=ADD,
                    )
                    nc.sync.dma_start(yout_ap[:, c0:c0 + 512], yb[:])

    nc.compile()
    return nc


def _ensure_ntff_hook():
    """Register the axon NTFF profiling hook if the image's antenv lacks it.

    Only used when tracing (DBN_TRACE); mirrors what trn_boot would register
    were antenv.axon_hooks present in the image.
    """
    try:
        import antenv.axon_hooks  # noqa: F401
        return
    except ImportError:
        pass
    try:
        import types

        import antenv
        from trn_agent_boot.trn_boot import _ntff_profile_via_ctypes

        hook = _ntff_profile_via_ctypes("/opt/axon/libaxon_pjrt.so")
        mod = types.ModuleType("antenv.axon_hooks")
        mod.get_axon_ntff_profile_hook = lambda: hook
        mod.set_axon_ntff_profile_hook = lambda h: None
        sys.modules["antenv.axon_hooks"] = mod
        antenv.axon_hooks = mod
    except Exception as e:  # profiling is best-effort
        print(f"ntff hook setup failed: {e}", file=sys.stderr)


def _run(x_flat, cols, ncores, trace=False):
    from concourse.bass_utils import run_bass_kernel_spmd

    if trace:
        _ensure_ntff_hook()

    nc = _build(cols, ncores)
    in_maps = [
        {"x": np.ascontiguousarray(x_flat[ROWS * k:ROWS * (k + 1)])}
        for k in range(ncores)
    ]
    res = run_bass_kernel_spmd(
        nc, in_maps, core_ids=list(range(ncores)), trace=trace
    )
    global LAST_RESULTS
    LAST_RESULTS = res
    return np.concatenate([r["y"] for r in res.results], axis=0)


def kernel(x: np.ndarray) -> np.ndarray:
    x = np.asarray(x)
    assert x.shape == (B, C, H, W) and x.dtype == np.float32
    xf = x.reshape(B * C, COLS)
    trace = bool(os.environ.get("DBN_TRACE"))
    yf = _run(xf, COLS, NCORES, trace=trace)
    return yf.reshape(B, C, H, W)


if __name__ == "__main__":
    xs = np.load("/tmp/ref_in.npy")
    ys = kernel(xs)
    expected = np.load("/tmp/ref_out.npy")
    rel = np.linalg.norm(ys - expected) / np.linalg.norm(expected)
    print("fro_rel:", rel)
    if LAST_RESULTS is not None:
        print("exec_time_ns:", LAST_RESULTS.exec_time_ns)
